# revision 1
# baseline (speedup 1.0000x reference)
"""CrossPSDLoss Trainium2 kernel.

Math (from the reference):
  res = target - pred; both [1024, 16384] f32.
  cross rows i=0..15: row i = concat_b x[b, 1024*i : 1024*(i+1)]  (length 1048576)
  Welch per row: 511 frames of 4096 (stride 2048), periodic-hann*2 window,
  rFFT, power, sum over frames -> S[k].  Loss only uses rows 8..15 and
  frequency bins 21..499 (the (20,500) mask with df=1), and the /T factors
  cancel in the ratio:
     out = (2/480) * sum_{row=8..15} sum_{kb=21..499} S_res[row,kb]/S_tgt[row,kb]

Sharding: one Welch row per NeuronCore (8 rows, 8 cores); each core consumes
only its [1024, 1024] column slice of pred/target.  No collectives; the host
sums the 8 per-core partial scalars.

Per-core pipeline:
  - host pre-casts the slice to bf16 (verified: final rel err ~1e-5)
  - DMA-transpose load -> XT[p, 1024*t + b] = X[b, 128*t + p]  (samples on
    partitions, which the TensorE contraction requires);
    frame_f[k] = XT[p, 1024*t + 2f + q] for k = 1024*q + 128*t + p = 128*j + p
  - res = tgt - pred on DVE (bf16)
  - even/odd fold (win/cos symmetric, sin antisymmetric about k=4096-k):
      u[k] = x[k] + x[4096-k],  v[k] = x[k] - x[4096-k],  k = 0..2047
      Re[n,f] = sum_{k=0..2047} C[k,n] u[k,f] + C[2048,n] x[2048,f]
      Im[n,f] = sum_{k=0..2047} S[k,n] v[k,f]
    built per 128-k-tile j=0..15 as psB = J0 @ B_j (+ row-0 partner
    mini-matmul), U_j = A_j + psB, V_j = A_j - psB on DVE, where
    A_j = y_j, B_j = y_{31-j}, J0 = anti-identity with row 0 zeroed.
    This HALVES the DFT GEMM contraction (16 k-tiles instead of 32).
  - windowed DFT GEMMs vs precomputed folded weights (bins 21..499 only),
    psum [chunk<=120, 511 frames]
  - PSD: Square activation with accum over frames, ratio + reduce on device.
"""

import os
import sys
from contextlib import ExitStack

import numpy as np
import ml_dtypes

for _p in ("/opt/trn_rl_repo", "/root/.axon_site/_ro/trn_rl_repo"):
    if os.path.isdir(_p) and _p not in sys.path:
        sys.path.insert(0, _p)

import concourse.bass as bass
import concourse.mybir as mybir
from concourse import bacc, tile
from concourse.bass_utils import run_bass_kernel_spmd

BF16 = ml_dtypes.bfloat16

NPERSEG = 4096
NSEG = 511
NBINS = 479          # bins 21..499
CHUNKS = [120, 120, 120, 119]   # 479 split into 4 partition chunks
N_CORES = 8
ROW0 = 8             # first Welch row that matters


def _y_ap(xtile, m):
    """AP of y_m[p, f] = frame_f[128*m + p] = XT[p, 1024*t + 2f + q],
    m = 8*q + t, for all 128 partitions and f = 0..510."""
    q, t = divmod(m, 8)
    base = 1024 * t + q
    return xtile[:, base: base + 1021: 2]


def _y0_ap(xtile, m):
    """Row-0 slice of _y_ap(xtile, m); also valid for m == 32 (q=4, t=0),
    whose weight row is zero."""
    q, t = divmod(m, 8)
    base = 1024 * t + q
    return xtile[0:1, base: base + 1021: 2]


def _build_nc() -> bass.Bass:
    # Bacc (not bass.Bass): its compile() runs generate_event_semaphores(),
    # which splits multi-semaphore waits into event-sem chains — TRN2
    # instructions support at most one wait each.
    nc = bacc.Bacc("TRN2", target_bir_lowering=False, debug=False,
                   num_devices=N_CORES)
    dt = mybir.dt

    # x inputs arrive t-major ([t, b, p] with p = column-within-128-block) so
    # every DMA-transpose reads a fully contiguous source (~350 GB/s instead
    # of the ~261 GB/s non-contiguous-mid-dim rate).
    xp_d = nc.dram_tensor("xp", [8, 1024, 128], dt.bfloat16,
                          kind="ExternalInput")
    xt_d = nc.dram_tensor("xt", [8, 1024, 128], dt.bfloat16,
                          kind="ExternalInput")
    wu_d = nc.dram_tensor("wu", [128, 16, NBINS], dt.bfloat16,
                          kind="ExternalInput")
    wv_d = nc.dram_tensor("wv", [128, 16, NBINS], dt.bfloat16,
                          kind="ExternalInput")
    wj0_d = nc.dram_tensor("wj0", [128, 128], dt.bfloat16,
                           kind="ExternalInput")
    w2k_d = nc.dram_tensor("w2k", [1, NBINS], dt.bfloat16,
                           kind="ExternalInput")
    out_d = nc.dram_tensor("out", [1, 1], dt.float32, kind="ExternalOutput")

    with ExitStack() as ctx:
        tc = ctx.enter_context(tile.TileContext(nc))
        xpool = ctx.enter_context(tc.tile_pool(name="x", bufs=1))
        wpool = ctx.enter_context(tc.tile_pool(name="w", bufs=1))
        uvpool = ctx.enter_context(tc.tile_pool(name="uv", bufs=1))
        psb = ctx.enter_context(tc.tile_pool(name="psb", bufs=4, space="PSUM"))
        pspool = ctx.enter_context(tc.tile_pool(name="ps", bufs=3, space="PSUM"))
        ps1 = ctx.enter_context(tc.tile_pool(name="ps1", bufs=1, space="PSUM"))
        scpool = ctx.enter_context(tc.tile_pool(name="sc", bufs=4))
        stat = ctx.enter_context(tc.tile_pool(name="stat", bufs=1))

        wu_sb = wpool.tile([128, 16, NBINS], dt.bfloat16, tag="wu")
        wv_sb = wpool.tile([128, 16, NBINS], dt.bfloat16, tag="wv")
        j0_sb = wpool.tile([128, 128], dt.bfloat16, tag="wj0")
        w2k_sb = wpool.tile([1, NBINS], dt.bfloat16, tag="w2k")
        xt_t = xpool.tile([128, 8192], dt.bfloat16, tag="xt_t")
        xp_t = xpool.tile([128, 8192], dt.bfloat16, tag="xp_t")
        xr_t = xpool.tile([128, 8192], dt.bfloat16, tag="xr_t")

        # DMA order = PE need order: xt tiles + J0 unblock the fold phase of
        # the tgt input first, then the GEMM weights, then xp for res.
        nc.sync.dma_start(j0_sb[:, :], wj0_d[:, :])
        nc.sync.dma_start(w2k_sb[:, :], w2k_d[:, :])
        for t in range(8):
            sl = slice(1024 * t, 1024 * (t + 1))
            nc.sync.dma_start(xt_t[:, sl], xt_d[t], transpose=True)
        nc.sync.dma_start(wu_sb[:, :, :], wu_d[:, :, :])
        nc.sync.dma_start(wv_sb[:, :, :], wv_d[:, :, :])
        for t in range(8):
            sl = slice(1024 * t, 1024 * (t + 1))
            nc.sync.dma_start(xp_t[:, sl], xp_d[t], transpose=True)
        for t in range(8):
            sl = slice(1024 * t, 1024 * (t + 1))
            nc.vector.tensor_sub(xr_t[:, sl], xt_t[:, sl], xp_t[:, sl])

        RATIO = stat.tile([128, 4], dt.float32)
        nc.vector.memset(RATIO[:, :], 0.0)
        ones = stat.tile([128, 1], dt.float32)
        nc.vector.memset(ones[:, :], 1.0)
        # e0: [1, 128] unit row vector; e0.T @ y0 writes y0 into psum row 0
        # and zeros rows 1..127 (full-region group open for the J0 matmul).
        e0 = stat.tile([1, 128], dt.bfloat16)
        nc.vector.memset(e0[:, :], 0.0)
        nc.vector.memset(e0[0:1, 0:1], 1.0)

        # Fold (both inputs first, so the PE's J0 matmuls for input 2 hide
        # the DVE U/V builds of input 1):
        #   psB_j = J0 @ y_{31-j}  (+ row-0 partner y_{32-j}[0]),
        #   U_j = y_j + psB_j, V_j = y_j - psB_j  (bf16, on DVE).
        UV = {}
        for xi, xtile in ((1, xt_t), (0, xr_t)):
            U = []
            V = []
            for j in range(16):
                pb = psb.tile([128, NSEG], dt.float32, tag="psB")
                # Row-0 partner first (e0.T @ y0 — full-region, opens the
                # group), then the J0 matmul closes it: J0's row 0 is
                # all-zero, so it accumulates 0 onto the partner row.
                nc.tensor.matmul(pb[:, :], e0[:, :],
                                 _y0_ap(xtile, 32 - j),
                                 start=True, stop=False)
                nc.tensor.matmul(pb[:, :], j0_sb[:, :], _y_ap(xtile, 31 - j),
                                 start=False, stop=True)
                u = uvpool.tile([128, NSEG], dt.bfloat16, tag=f"U{xi}_{j}")
                v = uvpool.tile([128, NSEG], dt.bfloat16, tag=f"V{xi}_{j}")
                # Bounce psB to SBUF bf16 on ACT so the DVE add/sub run in
                # 2x bf16 mode instead of 1x against fp32 PSUM.
                pbs = scpool.tile([128, NSEG], dt.bfloat16, tag="pbs")
                nc.scalar.copy(pbs[:, :], pb[:, :])
                nc.vector.tensor_add(u[:, :], _y_ap(xtile, j), pbs[:, :])
                nc.vector.tensor_sub(v[:, :], _y_ap(xtile, j), pbs[:, :])
                U.append(u)
                V.append(v)
            UV[xi] = (U, V)

        # E[(xi, trig, c)]: per-bin sum over the 511 frames of out^2 for
        # chunk c of the {cos,sin} DFT of input xi (0=res, 1=tgt).
        E = {}
        for xi, xtile in ((1, xt_t), (0, xr_t)):
            U, V = UV[xi]
            for m in range(8):
                c = m % 4
                trig = m // 4
                rows = CHUNKS[c]
                col0 = 120 * c
                w_sb = wu_sb if trig == 0 else wv_sb
                tiles = U if trig == 0 else V
                ps = pspool.tile([128, NSEG], dt.float32, tag="gemm_ps")
                for j in range(16):
                    nc.tensor.matmul(
                        ps[:rows, :],
                        w_sb[:, j, col0:col0 + rows],
                        tiles[j][:, :],
                        start=(j == 0),
                        stop=(trig == 1 and j == 15),
                    )
                if trig == 0:
                    # k = 2048 singleton (sin weight there is 0)
                    nc.tensor.matmul(
                        ps[:rows, :],
                        w2k_sb[:, col0:col0 + rows],
                        _y0_ap(xtile, 16),
                        start=False, stop=True)
                tmp = scpool.tile([128, NSEG], dt.float32, tag="sq")
                acc = stat.tile([128, 1], dt.float32, tag=f"E{xi}_{m}")
                E[(xi, trig, c)] = acc
                nc.scalar.activation(
                    out=tmp[:rows, :],
                    in_=ps[:rows, :],
                    func=mybir.ActivationFunctionType.Square,
                    accum_out=acc[:rows, :],
                )

        for c in range(4):
            rows = CHUNKS[c]
            sr = stat.tile([128, 1], dt.float32, tag=f"SR{c}")
            st = stat.tile([128, 1], dt.float32, tag=f"ST{c}")
            rec = stat.tile([128, 1], dt.float32, tag=f"REC{c}")
            nc.vector.tensor_add(sr[:rows, :], E[(0, 0, c)][:rows, :],
                                 E[(0, 1, c)][:rows, :])
            nc.vector.tensor_add(st[:rows, :], E[(1, 0, c)][:rows, :],
                                 E[(1, 1, c)][:rows, :])
            nc.vector.reciprocal(rec[:rows, :], st[:rows, :])
            nc.vector.tensor_mul(RATIO[:rows, c:c + 1], sr[:rows, :],
                                 rec[:rows, :])

        tot = ps1.tile([1, 4], dt.float32)
        nc.tensor.matmul(tot[:1, :4], ones[:, :1], RATIO[:, :4],
                         start=True, stop=True)
        scaled = stat.tile([1, 4], dt.float32)
        nc.vector.tensor_scalar_mul(scaled[:1, :], tot[:1, :], 2.0 / 480.0)
        red = stat.tile([1, 1], dt.float32)
        nc.vector.tensor_reduce(red[:1, :1], scaled[:1, :],
                                axis=mybir.AxisListType.X,
                                op=mybir.AluOpType.add)
        nc.sync.dma_start(out_d[:, :], red[:1, :1])

    nc.compile()
    return nc


def _build_w():
    """Folded DFT weights, all bf16:
      wu[p, j, n] = win[k] cos(2 pi k kb_n / 4096), k = 128 j + p  (u weights)
      wv[p, j, n] = win[k] sin(...)                                (v weights)
      wj0 = anti-identity J0[p, 128-p] = 1 for p = 1..127, row 0 zero
      w2k[0, n]  = win[2048] cos(2 pi 2048 kb_n / 4096)
    """
    k = np.arange(NPERSEG, dtype=np.float64)
    win = (0.5 - 0.5 * np.cos(2.0 * np.pi * k / NPERSEG)) * 2.0
    kb = np.arange(21, 21 + NBINS, dtype=np.float64)
    ang = 2.0 * np.pi * np.outer(k, kb) / NPERSEG
    C = win[:, None] * np.cos(ang)
    S = win[:, None] * np.sin(ang)
    wu = np.ascontiguousarray(
        C[:2048].reshape(16, 128, NBINS).transpose(1, 0, 2)).astype(BF16)
    wv = np.ascontiguousarray(
        S[:2048].reshape(16, 128, NBINS).transpose(1, 0, 2)).astype(BF16)
    j0 = np.zeros((128, 128), np.float64)
    for p in range(1, 128):
        j0[p, 128 - p] = 1.0
    w2k = np.ascontiguousarray(C[2048:2049]).astype(BF16)
    return {
        "wu": wu,
        "wv": wv,
        "wj0": j0.astype(BF16),
        "w2k": w2k,
    }


_CACHE: dict = {}


def _get_prog():
    if "nc" not in _CACHE:
        _CACHE["nc"] = _build_nc()
    return _CACHE["nc"]


def _get_w():
    if "w" not in _CACHE:
        _CACHE["w"] = _build_w()
    return _CACHE["w"]


def kernel(pred: np.ndarray, target: np.ndarray, _trace: bool = False):
    nc = _get_prog()
    w = _get_w()
    pred = np.asarray(pred)
    target = np.asarray(target)
    in_maps = []
    for i in range(N_CORES):
        c0 = (ROW0 + i) * 1024
        in_maps.append({
            "xp": np.ascontiguousarray(
                pred[:, c0:c0 + 1024].astype(BF16)
                .reshape(1024, 8, 128).transpose(1, 0, 2)),
            "xt": np.ascontiguousarray(
                target[:, c0:c0 + 1024].astype(BF16)
                .reshape(1024, 8, 128).transpose(1, 0, 2)),
            **w,
        })
    res = run_bass_kernel_spmd(nc, in_maps, list(range(N_CORES)), trace=_trace)
    total = float(sum(float(res.results[i]["out"][0, 0])
                      for i in range(N_CORES)))
    out = np.array(total, dtype=np.float32)
    if _trace:
        return out, res
    return out



# revision 20
# speedup vs baseline: 2.5343x; 2.5343x over previous
"""CrossPSDLoss Trainium2 kernel (fp8 DoubleRow direct-DFT version).

Math (from the reference):
  res = target - pred; both [1024, 16384] f32.
  cross rows i=0..15: row i = concat_b x[b, 1024*i : 1024*(i+1)]  (length 1048576)
  Welch per row: 511 frames of 4096 (stride 2048), periodic-hann*2 window,
  rFFT, power, sum over frames -> S[k].  Loss only uses rows 8..15 and
  frequency bins 21..499, and the /T factors cancel in the ratio:
     out = (2/480) * sum_{row=8..15} sum_{kb=21..499} S_res[row,kb]/S_tgt[row,kb]

Sharding: one Welch row per NeuronCore (8 rows, 8 cores); each core consumes
only its [1024, 1024] column slice of pred/target.  No collectives; the host
sums the 8 per-core partial scalars.

Per-core pipeline (vs the bf16 folded baseline; each step validated):
  - fp8(e4m3) everywhere on the DFT path; MatmulPerfMode.DoubleRow packs two
    128-deep k-tiles per matmul at 0.5 cycles/output-column -> 4x the bf16
    GEMM rate.  Walrus requires the stationary free size (2M) to be a
    multiple of 64 -> bin chunks of 128/96, not 120.  End-to-end rel err
    ~1.2e-4 (tol 2e-2).
  - NO even/odd fold (its U/V builds cost more DVE time than the PE time
    they save at fp8 rates): direct 32-k-tile contraction, 16 DoubleRow
    matmuls per (input, trig, bin-chunk) PSUM group.
  - Non-redundant x layout: frames overlap 50%, so sample (j,g) of the
    [128 part, 16 slot, 512 grp] buffer serves k-tile j at frame g AND
    k-tile j+16 at frame g-1 via a shifted AP.  Halves x DMA.
  - res transform by linearity in PSUM: DFT(res) = DFT(tgt) - DFT(pred),
    evaluated as (copy of tgt PSUM in SBUF) - (pred PSUM) on DVE (the
    engine can read at most one PSUM operand per instruction).
  - Every SBUF tile is written by exactly ONE DMA (x and weight tensors
    split into lo/hi tiles): the tile dependency tracker otherwise hangs
    first-use matmuls on later unrelated DMAs.
  - All 8 tgt GEMM groups run first (their weights stream during them),
    pred groups after, with xsp DMA'd last: minimizes PE stalls.
  - PE p-state warmup: dummy matmuls during the DMA lead-in so the real
    GEMMs run at 2.4 GHz from the first instruction.
  - Host prep is layout/dtype only (slice, reshape, transpose, fp8 cast).
"""

import os
import sys
from contextlib import ExitStack

import numpy as np
import ml_dtypes

for _p in ("/opt/trn_rl_repo", "/root/.axon_site/_ro/trn_rl_repo"):
    if os.path.isdir(_p) and _p not in sys.path:
        sys.path.insert(0, _p)

import concourse.bass as bass
import concourse.mybir as mybir
from concourse import bacc, tile
from concourse.bass_utils import run_bass_kernel_spmd

FP8 = ml_dtypes.float8_e4m3

NPERSEG = 4096
NSEG = 511
NBINS = 479          # bins 21..499
# Dual-fp8 ldweights requires the stationary free size (2M) to be a
# multiple of 64, so bin-chunks must be multiples of 32: pad 479 bins to
# 480 = 128*3 + 96 with one all-zero weight column; RCHUNKS excludes the
# pad bin from the ratio stage (its Et is 0 -> 1/0 would NaN the sum).
CHUNKS = [128, 128, 128, 96]
RCHUNKS = [128, 128, 128, 95]
N_CORES = 8
ROW0 = 8             # first Welch row that matters

# x-buffer slot s (0..15) holds k-tile s; k-tiles 16..31 alias slots 0..15
# shifted one frame.  Weight slot s holds k-tile PERM[s]: the first 16
# weight slots pair with x slots 0..7 (both plain and shifted), so the
# lo half of each GEMM group touches only lo-half tiles.
PERM = list(range(0, 8)) + list(range(16, 24)) + list(range(8, 16)) + list(range(24, 32))
_SLOT_OF = {t: s for s, t in enumerate(PERM)}
# matmul issue order: lo block (x slots 0..7), then hi block
KK_ORDER = [0, 1, 2, 3, 8, 9, 10, 11, 4, 5, 6, 7, 12, 13, 14, 15]

N_WARMUP = int(os.environ.get("KERNEL_N_WARMUP", "42"))


def _build_nc() -> bass.Bass:
    # Bacc (not bass.Bass): its compile() runs generate_event_semaphores(),
    # which splits multi-semaphore waits into event-sem chains — TRN2
    # instructions support at most one wait each.
    nc = bacc.Bacc("TRN2", target_bir_lowering=False, debug=False,
                   num_devices=N_CORES)
    dt = mybir.dt
    DR = mybir.MatmulPerfMode.DoubleRow

    xst_d = nc.dram_tensor("xst", [128, 16, 512], dt.float8e4,
                           kind="ExternalInput")
    xsp_d = nc.dram_tensor("xsp", [128, 16, 512], dt.float8e4,
                           kind="ExternalInput")
    wd = {}
    for trig in ("c", "s"):
        for ci, rows in enumerate(CHUNKS):
            nm = f"w{trig}{ci}"
            wd[nm] = nc.dram_tensor(nm, [128, 32, rows], dt.float8e4,
                                    kind="ExternalInput")
    out_d = nc.dram_tensor("out", [1, 1], dt.float32, kind="ExternalOutput")

    with ExitStack() as ctx:
        tc = ctx.enter_context(tile.TileContext(nc))
        xpool = ctx.enter_context(tc.tile_pool(name="x", bufs=1))
        wpool = ctx.enter_context(tc.tile_pool(name="w", bufs=1))
        pst_pool = ctx.enter_context(tc.tile_pool(name="pst", bufs=3, space="PSUM"))
        psp_pool = ctx.enter_context(tc.tile_pool(name="psp", bufs=3, space="PSUM"))
        ps1 = ctx.enter_context(tc.tile_pool(name="ps1", bufs=1, space="PSUM"))
        ptpool = ctx.enter_context(tc.tile_pool(name="pt", bufs=1))
        dpool = ctx.enter_context(tc.tile_pool(name="d", bufs=2))
        scpool = ctx.enter_context(tc.tile_pool(name="sc", bufs=3))
        stat = ctx.enter_context(tc.tile_pool(name="stat", bufs=1))

        # PE p-state warmup (see module docstring).
        if N_WARMUP:
            wa = stat.tile([1, 128], dt.bfloat16)
            nc.vector.memset(wa[:, :], 1.0)
            wps = ps1.tile([128, 128], dt.float32)
            for _ in range(N_WARMUP):
                nc.tensor.matmul(wps[:, :], wa[:1, :], wa[:1, :],
                                 start=True, stop=True)
            warm_junk = stat.tile([1, 1], dt.float32)
            nc.vector.tensor_copy(warm_junk[:1, :1], wps[0:1, 0:1])

        # lo/hi tiles: one DMA per tile
        xs = {}
        for nm, dram in (("t", xst_d), ("p", xsp_d)):
            xs[(nm, 0)] = xpool.tile([128, 8, 512], dt.float8e4,
                                     tag=f"xs{nm}lo", name=f"xs{nm}lo")
            xs[(nm, 1)] = xpool.tile([128, 8, 512], dt.float8e4,
                                     tag=f"xs{nm}hi", name=f"xs{nm}hi")
        wsb = {}
        for trig in ("c", "s"):
            for ci, rows in enumerate(CHUNKS):
                for half in (0, 1):
                    nm = f"w{trig}{ci}{'lo' if half == 0 else 'hi'}"
                    wsb[(trig, ci, half)] = wpool.tile(
                        [128, 16, rows], dt.float8e4, tag=nm, name=nm)

        def dma_w(trig, ci):
            rows = CHUNKS[ci]
            for half in (0, 1):
                nc.sync.dma_start(wsb[(trig, ci, half)][:, :, :],
                                  wd[f"w{trig}{ci}"][:, 16 * half:16 * half + 16, :])

        def dma_x(nm, dram):
            for half in (0, 1):
                nc.sync.dma_start(xs[(nm, half)][:, :, :],
                                  dram[:, 8 * half:8 * half + 8, :])

        # DMA order = PE need order (tgt groups c0..c3,s0..s3, then pred).
        dma_w("c", 0)
        dma_x("t", xst_d)
        dma_w("c", 1)
        dma_w("c", 2)
        dma_w("c", 3)
        dma_w("s", 0)
        dma_w("s", 1)
        dma_w("s", 2)
        dma_w("s", 3)
        dma_x("p", xsp_d)

        def gemm(ps_t, inp, trig, ci, f_lo=0, f_hi=NSEG, start=True, stop=True):
            """One PSD transform over frames [f_lo, f_hi): 16 DoubleRow
            matmuls (32 k-tiles of 128), lo-half tiles first."""
            rows = CHUNKS[ci]
            n = f_hi - f_lo
            for pos, kk in enumerate(KK_ORDER):
                t0 = 2 * kk
                s0 = _SLOT_OF[t0]
                xtile = xs[(inp, 0 if t0 % 16 < 8 else 1)]
                if t0 < 16:
                    rhs = xtile[:, (t0 % 8):(t0 % 8) + 2, f_lo:f_hi]
                else:
                    rhs = xtile[:, (t0 % 8):(t0 % 8) + 2, f_lo + 1:f_hi + 1]
                wtile = wsb[(trig, ci, 0 if s0 < 16 else 1)]
                nc.tensor.matmul(
                    ps_t[:rows, f_lo:f_hi],
                    wtile[:, s0 % 16:s0 % 16 + 2, :rows],
                    rhs,
                    start=(start and pos == 0),
                    stop=(stop and pos == 15),
                    perf_mode=DR,
                )

        GROUPS = [(t, c) for t in ("c", "s") for c in range(4)]

        # Phase 1: all 8 tgt transforms; each PSUM is copied to SBUF (pt_sb)
        # then squared+accumulated (Et) off the PE critical path.
        PT = {}
        E = {}
        for trig, ci in GROUPS:
            rows = CHUNKS[ci]
            ps_t = pst_pool.tile([128, NSEG], dt.float32, tag="ps_t")
            gemm(ps_t, "t", trig, ci)
            pt_sb = ptpool.tile([128, NSEG], dt.float32,
                                tag=f"pt_{trig}{ci}", name=f"pt_{trig}{ci}")
            nc.vector.tensor_copy(pt_sb[:rows, :], ps_t[:rows, :])
            PT[(trig, ci)] = pt_sb
            et = stat.tile([128, 1], dt.float32, tag=f"Et_{trig}{ci}",
                           name=f"Et_{trig}{ci}")
            tmp_t = scpool.tile([128, NSEG], dt.float32, tag="sq_t")
            nc.scalar.activation(
                out=tmp_t[:rows, :], in_=pt_sb[:rows, :],
                func=mybir.ActivationFunctionType.Square,
                accum_out=et[:rows, :])
            E[(1, trig, ci)] = et

        # Denominator sums + reciprocals (all off the critical path once
        # phase 1 is done).
        REC = {}
        for trig_unused in (None,):
            for ci, rows in enumerate(RCHUNKS):
                st_ = stat.tile([128, 1], dt.float32, tag=f"ST{ci}",
                                name=f"ST{ci}")
                rec = stat.tile([128, 1], dt.float32, tag=f"REC{ci}",
                                name=f"REC{ci}")
                nc.vector.tensor_add(st_[:rows, :], E[(1, "c", ci)][:rows, :],
                                     E[(1, "s", ci)][:rows, :])
                nc.vector.reciprocal(rec[:rows, :], st_[:rows, :])
                REC[ci] = rec

        # Phase 2: pred transforms; d = pt_sb - ps_p (one PSUM operand),
        # Er = sum_f d^2 via ACT Square+accum.
        for gi, (trig, ci) in enumerate(GROUPS):
            rows = CHUNKS[ci]
            ps_p = psp_pool.tile([128, NSEG], dt.float32, tag="ps_p")
            gemm(ps_p, "p", trig, ci)
            pt_sb = PT[(trig, ci)]
            d = dpool.tile([128, NSEG], dt.float32, tag="d")
            nc.vector.tensor_sub(d[:rows, :], pt_sb[:rows, :], ps_p[:rows, :])
            er = stat.tile([128, 1], dt.float32, tag=f"Er_{trig}{ci}",
                           name=f"Er_{trig}{ci}")
            tmp_r = scpool.tile([128, NSEG], dt.float32, tag="sq_r")
            nc.scalar.activation(
                out=tmp_r[:rows, :], in_=d[:rows, :],
                func=mybir.ActivationFunctionType.Square,
                accum_out=er[:rows, :])
            E[(0, trig, ci)] = er

        # Finale: sum_bins (Er_c+Er_s) * rec as PE dot products accumulated
        # into one PSUM scalar (partition-dim reduce for free).
        tot = ps1.tile([1, 1], dt.float32)
        for ci, rows in enumerate(RCHUNKS):
            sr = stat.tile([128, 1], dt.float32, tag=f"SR{ci}", name=f"SR{ci}")
            nc.vector.tensor_add(sr[:rows, :], E[(0, "c", ci)][:rows, :],
                                 E[(0, "s", ci)][:rows, :])
            nc.tensor.matmul(tot[:1, :1], sr[:rows, :1], REC[ci][:rows, :1],
                             start=(ci == 0), stop=(ci == 3))
        red = stat.tile([1, 1], dt.float32)
        nc.vector.tensor_scalar_mul(red[:1, :1], tot[:1, :1], 2.0 / 480.0)
        nc.sync.dma_start(out_d[:, :], red[:1, :1])

    nc.compile()
    return nc


def _build_w():
    """fp8 DFT weights in the [part, slot, bin-chunk] layout:
      w{c,s}{ci}[p, s, n] = win[k] * {cos,sin}(2 pi k (21 + n0 + n) / 4096),
      k = 128 * PERM[s] + p.
    """
    k = np.arange(NPERSEG, dtype=np.float64)
    win = (0.5 - 0.5 * np.cos(2.0 * np.pi * k / NPERSEG)) * 2.0
    kb = np.arange(21, 21 + NBINS, dtype=np.float64)
    ang = 2.0 * np.pi * np.outer(k, kb) / NPERSEG
    nb_pad = sum(CHUNKS)
    C = np.zeros((NPERSEG, nb_pad), np.float32)
    S = np.zeros((NPERSEG, nb_pad), np.float32)
    C[:, :NBINS] = win[:, None] * np.cos(ang)
    S[:, :NBINS] = win[:, None] * np.sin(ang)
    out = {}
    for trig, M in (("c", C), ("s", S)):
        # [4096, 480] -> [p, j, n] -> permute j into slots
        M3 = M.reshape(32, 128, nb_pad).transpose(1, 0, 2)[:, PERM, :]
        col0 = 0
        for ci, rows in enumerate(CHUNKS):
            out[f"w{trig}{ci}"] = np.ascontiguousarray(
                M3[:, :, col0:col0 + rows]).astype(FP8)
            col0 += rows
    return out


_CACHE: dict = {}


def _get_prog():
    if "nc" not in _CACHE:
        _CACHE["nc"] = _build_nc()
    return _CACHE["nc"]


def _get_w():
    if "w" not in _CACHE:
        _CACHE["w"] = _build_w()
    return _CACHE["w"]


def _xs_layout(x: np.ndarray, core: int) -> np.ndarray:
    """[128 part, 16 slot, 512 grp] fp8 view of Welch row ROW0+core:
    xs[p, j, g] = R[2048 g + 128 j + p] where R is the row's 1048576
    samples (R[1024 b + m] = x[b, 1024 (ROW0+core) + m])."""
    c0 = (ROW0 + core) * 1024
    R = np.ascontiguousarray(x[:, c0:c0 + 1024]).reshape(-1).astype(FP8)
    return np.ascontiguousarray(R.reshape(512, 16, 128).transpose(2, 1, 0))


def kernel(pred: np.ndarray, target: np.ndarray, _trace: bool = False):
    nc = _get_prog()
    w = _get_w()
    pred = np.asarray(pred)
    target = np.asarray(target)
    in_maps = []
    for i in range(N_CORES):
        in_maps.append({
            "xst": _xs_layout(target, i),
            "xsp": _xs_layout(pred, i),
            **w,
        })
    res = run_bass_kernel_spmd(nc, in_maps, list(range(N_CORES)), trace=_trace)
    total = float(sum(float(res.results[i]["out"][0, 0])
                      for i in range(N_CORES)))
    out = np.array(total, dtype=np.float32)
    if _trace:
        return out, res
    return out


# revision 26
# speedup vs baseline: 2.5598x; 1.0101x over previous
"""CrossPSDLoss Trainium2 kernel (fp8 DoubleRow direct-DFT version).

Math (from the reference):
  res = target - pred; both [1024, 16384] f32.
  cross rows i=0..15: row i = concat_b x[b, 1024*i : 1024*(i+1)]  (length 1048576)
  Welch per row: 511 frames of 4096 (stride 2048), periodic-hann*2 window,
  rFFT, power, sum over frames -> S[k].  Loss only uses rows 8..15 and
  frequency bins 21..499, and the /T factors cancel in the ratio:
     out = (2/480) * sum_{row=8..15} sum_{kb=21..499} S_res[row,kb]/S_tgt[row,kb]

Sharding: one Welch row per NeuronCore (8 rows, 8 cores); each core consumes
only its [1024, 1024] column slice of pred/target.  No collectives; the host
sums the 8 per-core partial scalars.

Per-core pipeline (vs the bf16 folded baseline; each step validated):
  - fp8(e4m3) everywhere on the DFT path; MatmulPerfMode.DoubleRow packs two
    128-deep k-tiles per matmul at 0.5 cycles/output-column -> 4x the bf16
    GEMM rate.  Walrus requires the stationary free size (2M) to be a
    multiple of 64 -> bin chunks of 128/96, not 120.  End-to-end rel err
    ~1.2e-4 (tol 2e-2).
  - NO even/odd fold (its U/V builds cost more DVE time than the PE time
    they save at fp8 rates): direct 32-k-tile contraction, 16 DoubleRow
    matmuls per (input, trig, bin-chunk) PSUM group.
  - Non-redundant x layout: frames overlap 50%, so sample (j,g) of the
    [128 part, 16 slot, 512 grp] buffer serves k-tile j at frame g AND
    k-tile j+16 at frame g-1 via a shifted AP.  Halves x DMA.
  - res transform by linearity in PSUM: DFT(res) = DFT(tgt) - DFT(pred),
    evaluated as (copy of tgt PSUM in SBUF) - (pred PSUM) on DVE (the
    engine can read at most one PSUM operand per instruction).
  - Every SBUF tile is written by exactly ONE DMA (x and weight tensors
    split into lo/hi tiles): the tile dependency tracker otherwise hangs
    first-use matmuls on later unrelated DMAs.
  - All 8 tgt GEMM groups run first (their weights stream during them),
    pred groups after, with xsp DMA'd last: minimizes PE stalls.
  - PE p-state warmup: dummy matmuls during the DMA lead-in so the real
    GEMMs run at 2.4 GHz from the first instruction.
  - Host prep is layout/dtype only (slice, reshape, transpose, fp8 cast).
"""

import os
import sys
from contextlib import ExitStack

import numpy as np
import ml_dtypes

for _p in ("/opt/trn_rl_repo", "/root/.axon_site/_ro/trn_rl_repo"):
    if os.path.isdir(_p) and _p not in sys.path:
        sys.path.insert(0, _p)

import concourse.bass as bass
import concourse.mybir as mybir
from concourse import bacc, tile
from concourse.bass_utils import run_bass_kernel_spmd

FP8 = ml_dtypes.float8_e4m3

NPERSEG = 4096
NSEG = 511
NBINS = 479          # bins 21..499
# Dual-fp8 ldweights requires the stationary free size (2M) to be a
# multiple of 64, so bin-chunks must be multiples of 32: pad 479 bins to
# 480 = 128*3 + 96 with one all-zero weight column; RCHUNKS excludes the
# pad bin from the ratio stage (its Et is 0 -> 1/0 would NaN the sum).
CHUNKS = [128, 128, 128, 96]
RCHUNKS = [128, 128, 128, 95]
N_CORES = 8
ROW0 = 8             # first Welch row that matters

# x-buffer slot s (0..15) holds k-tile s; k-tiles 16..31 alias slots 0..15
# shifted one frame.  Weight slot s holds k-tile PERM[s]: the first 16
# weight slots pair with x slots 0..7 (both plain and shifted), so the
# lo half of each GEMM group touches only lo-half tiles.
PERM = list(range(0, 8)) + list(range(16, 24)) + list(range(8, 16)) + list(range(24, 32))
_SLOT_OF = {t: s for s, t in enumerate(PERM)}
# matmul issue order: lo block (x slots 0..7), then hi block
KK_ORDER = [0, 1, 2, 3, 8, 9, 10, 11, 4, 5, 6, 7, 12, 13, 14, 15]

N_WARMUP = int(os.environ.get("KERNEL_N_WARMUP", "20"))


def _build_nc() -> bass.Bass:
    # Bacc (not bass.Bass): its compile() runs generate_event_semaphores(),
    # which splits multi-semaphore waits into event-sem chains — TRN2
    # instructions support at most one wait each.
    nc = bacc.Bacc("TRN2", target_bir_lowering=False, debug=False,
                   num_devices=N_CORES)
    dt = mybir.dt
    DR = mybir.MatmulPerfMode.DoubleRow

    xst_d = nc.dram_tensor("xst", [128, 16, 512], dt.float8e4,
                           kind="ExternalInput")
    xsp_d = nc.dram_tensor("xsp", [128, 16, 512], dt.float8e4,
                           kind="ExternalInput")
    wd = {}
    for trig in ("c", "s"):
        for ci, rows in enumerate(CHUNKS):
            nm = f"w{trig}{ci}"
            wd[nm] = nc.dram_tensor(nm, [128, 32, rows], dt.float8e4,
                                    kind="ExternalInput")
    out_d = nc.dram_tensor("out", [1, 1], dt.float32, kind="ExternalOutput")

    with ExitStack() as ctx:
        tc = ctx.enter_context(tile.TileContext(nc))
        xpool = ctx.enter_context(tc.tile_pool(name="x", bufs=1))
        wpool = ctx.enter_context(tc.tile_pool(name="w", bufs=1))
        pst_pool = ctx.enter_context(tc.tile_pool(name="pst", bufs=3, space="PSUM"))
        psp_pool = ctx.enter_context(tc.tile_pool(name="psp", bufs=2, space="PSUM"))
        ps1 = ctx.enter_context(tc.tile_pool(name="ps1", bufs=1, space="PSUM"))
        psb = ctx.enter_context(tc.tile_pool(name="psb", bufs=1, space="PSUM"))
        ptpool = ctx.enter_context(tc.tile_pool(name="pt", bufs=1))
        dpool = ctx.enter_context(tc.tile_pool(name="d", bufs=2))
        scpool = ctx.enter_context(tc.tile_pool(name="sc", bufs=3))
        stat = ctx.enter_context(tc.tile_pool(name="stat", bufs=1))

        # PE p-state warmup (see module docstring).  256-wide so the engine
        # time per matmul (213 ns at the mid p-state) exceeds the PE.SEQ
        # dispatch cost (~142 ns for ldweights+matmult) — narrower warmups
        # throttle on the sequencer and delay the first real GEMM dispatch.
        if N_WARMUP:
            wa = stat.tile([1, 256], dt.bfloat16)
            nc.vector.memset(wa[:, :], 1.0)
            wps = ps1.tile([128, 256], dt.float32)
            for _ in range(N_WARMUP):
                nc.tensor.matmul(wps[:, :], wa[:1, :128], wa[:1, :],
                                 start=True, stop=True)
            warm_junk = stat.tile([1, 1], dt.float32)
            nc.vector.tensor_copy(warm_junk[:1, :1], wps[0:1, 0:1])

        # lo/hi tiles: one DMA per tile
        xs = {}
        for nm, dram in (("t", xst_d), ("p", xsp_d)):
            xs[(nm, 0)] = xpool.tile([128, 8, 512], dt.float8e4,
                                     tag=f"xs{nm}lo", name=f"xs{nm}lo")
            xs[(nm, 1)] = xpool.tile([128, 8, 512], dt.float8e4,
                                     tag=f"xs{nm}hi", name=f"xs{nm}hi")
        wsb = {}
        for trig in ("c", "s"):
            for ci, rows in enumerate(CHUNKS):
                for half in (0, 1):
                    nm = f"w{trig}{ci}{'lo' if half == 0 else 'hi'}"
                    wsb[(trig, ci, half)] = wpool.tile(
                        [128, 16, rows], dt.float8e4, tag=nm, name=nm)

        def dma_w(trig, ci):
            rows = CHUNKS[ci]
            for half in (0, 1):
                nc.sync.dma_start(wsb[(trig, ci, half)][:, :, :],
                                  wd[f"w{trig}{ci}"][:, 16 * half:16 * half + 16, :])

        def dma_x(nm, dram):
            for half in (0, 1):
                nc.sync.dma_start(xs[(nm, half)][:, :, :],
                                  dram[:, 8 * half:8 * half + 8, :])

        # DMA order = PE need order (tgt groups c0..c3,s0..s3, then pred).
        dma_w("c", 0)
        dma_x("t", xst_d)
        dma_w("c", 1)
        dma_w("c", 2)
        dma_w("c", 3)
        dma_w("s", 0)
        dma_w("s", 1)
        dma_w("s", 2)
        dma_w("s", 3)
        dma_x("p", xsp_d)

        def gemm(ps_t, inp, trig, ci, f_lo=0, f_hi=NSEG, out0=0):
            """One PSD transform over frames [f_lo, f_hi) into psum columns
            [out0, out0 + f_hi - f_lo): 16 DoubleRow matmuls (32 k-tiles of
            128), lo-half tiles first."""
            rows = CHUNKS[ci]
            n = f_hi - f_lo
            for pos, kk in enumerate(KK_ORDER):
                t0 = 2 * kk
                s0 = _SLOT_OF[t0]
                xtile = xs[(inp, 0 if t0 % 16 < 8 else 1)]
                if t0 < 16:
                    rhs = xtile[:, (t0 % 8):(t0 % 8) + 2, f_lo:f_hi]
                else:
                    rhs = xtile[:, (t0 % 8):(t0 % 8) + 2, f_lo + 1:f_hi + 1]
                wtile = wsb[(trig, ci, 0 if s0 < 16 else 1)]
                nc.tensor.matmul(
                    ps_t[:rows, out0:out0 + n],
                    wtile[:, s0 % 16:s0 % 16 + 2, :rows],
                    rhs,
                    start=(pos == 0),
                    stop=(pos == 15),
                    perf_mode=DR,
                )

        GROUPS = [(t, c) for t in ("c", "s") for c in range(4)]

        # Phase 1: all 8 tgt transforms; each PSUM is copied to SBUF (pt_sb)
        # then squared+accumulated (Et) off the PE critical path.
        PT = {}
        E = {}
        for trig, ci in GROUPS:
            rows = CHUNKS[ci]
            ps_t = pst_pool.tile([128, NSEG], dt.float32, tag="ps_t")
            gemm(ps_t, "t", trig, ci)
            pt_sb = ptpool.tile([128, NSEG], dt.float32,
                                tag=f"pt_{trig}{ci}", name=f"pt_{trig}{ci}")
            nc.vector.tensor_copy(pt_sb[:rows, :], ps_t[:rows, :])
            PT[(trig, ci)] = pt_sb
            et = stat.tile([128, 1], dt.float32, tag=f"Et_{trig}{ci}",
                           name=f"Et_{trig}{ci}")
            tmp_t = scpool.tile([128, NSEG], dt.float32, tag="sq_t")
            nc.scalar.activation(
                out=tmp_t[:rows, :], in_=pt_sb[:rows, :],
                func=mybir.ActivationFunctionType.Square,
                accum_out=et[:rows, :])
            E[(1, trig, ci)] = et

        # Denominator sums + reciprocals (all off the critical path once
        # phase 1 is done).
        REC = {}
        for trig_unused in (None,):
            for ci, rows in enumerate(RCHUNKS):
                st_ = stat.tile([128, 1], dt.float32, tag=f"ST{ci}",
                                name=f"ST{ci}")
                rec = stat.tile([128, 1], dt.float32, tag=f"REC{ci}",
                                name=f"REC{ci}")
                nc.vector.tensor_add(st_[:rows, :], E[(1, "c", ci)][:rows, :],
                                     E[(1, "s", ci)][:rows, :])
                nc.vector.reciprocal(rec[:rows, :], st_[:rows, :])
                REC[ci] = rec

        # Phase 2: pred transforms; d = pt_sb - ps_p (one PSUM operand),
        # Er = sum_f d^2 via ACT Square+accum.  The very last group is
        # frame-split (448 + 63) into two PSUM groups on separate banks so
        # most of its sub runs while the PE finishes the 63-frame remainder
        # (shortens the tail-critical chain).
        FSPLIT = 448
        for gi, (trig, ci) in enumerate(GROUPS):
            rows = CHUNKS[ci]
            last = (gi == len(GROUPS) - 1)
            ps_p = psp_pool.tile([128, NSEG], dt.float32, tag="ps_p")
            pt_sb = PT[(trig, ci)]
            d = dpool.tile([128, NSEG], dt.float32, tag="d")
            if last:
                gemm(ps_p, "p", trig, ci, 0, FSPLIT)
                ps_pb = psb.tile([128, NSEG - FSPLIT], dt.float32, tag="ps_pb")
                gemm(ps_pb, "p", trig, ci, FSPLIT, NSEG, out0=0)
                nc.vector.tensor_sub(d[:rows, :FSPLIT], pt_sb[:rows, :FSPLIT],
                                     ps_p[:rows, :FSPLIT])
                nc.vector.tensor_sub(d[:rows, FSPLIT:], pt_sb[:rows, FSPLIT:],
                                     ps_pb[:rows, :])
            else:
                gemm(ps_p, "p", trig, ci)
                nc.vector.tensor_sub(d[:rows, :], pt_sb[:rows, :],
                                     ps_p[:rows, :])
            er = stat.tile([128, 1], dt.float32, tag=f"Er_{trig}{ci}",
                           name=f"Er_{trig}{ci}")
            tmp_r = scpool.tile([128, NSEG], dt.float32, tag="sq_r")
            nc.scalar.activation(
                out=tmp_r[:rows, :], in_=d[:rows, :],
                func=mybir.ActivationFunctionType.Square,
                accum_out=er[:rows, :])
            E[(0, trig, ci)] = er

        # Finale: sum_bins (Er_c+Er_s) * rec as PE dot products accumulated
        # into one PSUM scalar (partition-dim reduce for free).
        tot = ps1.tile([1, 1], dt.float32)
        for ci, rows in enumerate(RCHUNKS):
            sr = stat.tile([128, 1], dt.float32, tag=f"SR{ci}", name=f"SR{ci}")
            nc.vector.tensor_add(sr[:rows, :], E[(0, "c", ci)][:rows, :],
                                 E[(0, "s", ci)][:rows, :])
            nc.tensor.matmul(tot[:1, :1], sr[:rows, :1], REC[ci][:rows, :1],
                             start=(ci == 0), stop=(ci == 3))
        red = stat.tile([1, 1], dt.float32)
        nc.vector.tensor_scalar_mul(red[:1, :1], tot[:1, :1], 2.0 / 480.0)
        nc.sync.dma_start(out_d[:, :], red[:1, :1])

    nc.compile()
    return nc


def _build_w():
    """fp8 DFT weights in the [part, slot, bin-chunk] layout:
      w{c,s}{ci}[p, s, n] = win[k] * {cos,sin}(2 pi k (21 + n0 + n) / 4096),
      k = 128 * PERM[s] + p.
    """
    k = np.arange(NPERSEG, dtype=np.float64)
    win = (0.5 - 0.5 * np.cos(2.0 * np.pi * k / NPERSEG)) * 2.0
    kb = np.arange(21, 21 + NBINS, dtype=np.float64)
    ang = 2.0 * np.pi * np.outer(k, kb) / NPERSEG
    nb_pad = sum(CHUNKS)
    C = np.zeros((NPERSEG, nb_pad), np.float32)
    S = np.zeros((NPERSEG, nb_pad), np.float32)
    C[:, :NBINS] = win[:, None] * np.cos(ang)
    S[:, :NBINS] = win[:, None] * np.sin(ang)
    out = {}
    for trig, M in (("c", C), ("s", S)):
        # [4096, 480] -> [p, j, n] -> permute j into slots
        M3 = M.reshape(32, 128, nb_pad).transpose(1, 0, 2)[:, PERM, :]
        col0 = 0
        for ci, rows in enumerate(CHUNKS):
            out[f"w{trig}{ci}"] = np.ascontiguousarray(
                M3[:, :, col0:col0 + rows]).astype(FP8)
            col0 += rows
    return out


_CACHE: dict = {}


def _get_prog():
    if "nc" not in _CACHE:
        _CACHE["nc"] = _build_nc()
    return _CACHE["nc"]


def _get_w():
    if "w" not in _CACHE:
        _CACHE["w"] = _build_w()
    return _CACHE["w"]


def _xs_layout(x: np.ndarray, core: int) -> np.ndarray:
    """[128 part, 16 slot, 512 grp] fp8 view of Welch row ROW0+core:
    xs[p, j, g] = R[2048 g + 128 j + p] where R is the row's 1048576
    samples (R[1024 b + m] = x[b, 1024 (ROW0+core) + m])."""
    c0 = (ROW0 + core) * 1024
    R = np.ascontiguousarray(x[:, c0:c0 + 1024]).reshape(-1).astype(FP8)
    return np.ascontiguousarray(R.reshape(512, 16, 128).transpose(2, 1, 0))


def kernel(pred: np.ndarray, target: np.ndarray, _trace: bool = False):
    nc = _get_prog()
    w = _get_w()
    pred = np.asarray(pred)
    target = np.asarray(target)
    in_maps = []
    for i in range(N_CORES):
        in_maps.append({
            "xst": _xs_layout(target, i),
            "xsp": _xs_layout(pred, i),
            **w,
        })
    res = run_bass_kernel_spmd(nc, in_maps, list(range(N_CORES)), trace=_trace)
    total = float(sum(float(res.results[i]["out"][0, 0])
                      for i in range(N_CORES)))
    out = np.array(total, dtype=np.float32)
    if _trace:
        return out, res
    return out


# revision 29
# speedup vs baseline: 2.5646x; 1.0019x over previous
"""CrossPSDLoss Trainium2 kernel (fp8 DoubleRow direct-DFT version).

Math (from the reference):
  res = target - pred; both [1024, 16384] f32.
  cross rows i=0..15: row i = concat_b x[b, 1024*i : 1024*(i+1)]  (length 1048576)
  Welch per row: 511 frames of 4096 (stride 2048), periodic-hann*2 window,
  rFFT, power, sum over frames -> S[k].  Loss only uses rows 8..15 and
  frequency bins 21..499, and the /T factors cancel in the ratio:
     out = (2/480) * sum_{row=8..15} sum_{kb=21..499} S_res[row,kb]/S_tgt[row,kb]

Sharding: one Welch row per NeuronCore (8 rows, 8 cores); each core consumes
only its [1024, 1024] column slice of pred/target.  No collectives; the host
sums the 8 per-core partial scalars.

Per-core pipeline (vs the bf16 folded baseline; each step validated):
  - fp8(e4m3) everywhere on the DFT path; MatmulPerfMode.DoubleRow packs two
    128-deep k-tiles per matmul at 0.5 cycles/output-column -> 4x the bf16
    GEMM rate.  Walrus requires the stationary free size (2M) to be a
    multiple of 64 -> bin chunks of 128/96, not 120.  End-to-end rel err
    ~1.2e-4 (tol 2e-2).
  - NO even/odd fold (its U/V builds cost more DVE time than the PE time
    they save at fp8 rates): direct 32-k-tile contraction, 16 DoubleRow
    matmuls per (input, trig, bin-chunk) PSUM group.
  - Non-redundant x layout: frames overlap 50%, so sample (j,g) of the
    [128 part, 16 slot, 512 grp] buffer serves k-tile j at frame g AND
    k-tile j+16 at frame g-1 via a shifted AP.  Halves x DMA.
  - res transform by linearity in PSUM: DFT(res) = DFT(tgt) - DFT(pred),
    evaluated as (copy of tgt PSUM in SBUF) - (pred PSUM) on DVE (the
    engine can read at most one PSUM operand per instruction).
  - Every SBUF tile is written by exactly ONE DMA (x and weight tensors
    split into lo/hi tiles): the tile dependency tracker otherwise hangs
    first-use matmuls on later unrelated DMAs.
  - All 8 tgt GEMM groups run first (their weights stream during them),
    pred groups after, with xsp DMA'd last: minimizes PE stalls.
  - PE p-state warmup: dummy matmuls during the DMA lead-in so the real
    GEMMs run at 2.4 GHz from the first instruction.
  - Host prep is layout/dtype only (slice, reshape, transpose, fp8 cast).
"""

import os
import sys
from contextlib import ExitStack

import numpy as np
import ml_dtypes

for _p in ("/opt/trn_rl_repo", "/root/.axon_site/_ro/trn_rl_repo"):
    if os.path.isdir(_p) and _p not in sys.path:
        sys.path.insert(0, _p)

import concourse.bass as bass
import concourse.mybir as mybir
from concourse import bacc, tile
from concourse.bass_utils import run_bass_kernel_spmd

FP8 = ml_dtypes.float8_e4m3

NPERSEG = 4096
NSEG = 511
NBINS = 479          # bins 21..499
# Dual-fp8 ldweights requires the stationary free size (2M) to be a
# multiple of 64, so bin-chunks must be multiples of 32: pad 479 bins to
# 480 = 128*3 + 96 with one all-zero weight column; RCHUNKS excludes the
# pad bin from the ratio stage (its Et is 0 -> 1/0 would NaN the sum).
CHUNKS = [128, 128, 128, 96]
RCHUNKS = [128, 128, 128, 95]
N_CORES = 8
ROW0 = 8             # first Welch row that matters

# x-buffer slot s (0..15) holds k-tile s; k-tiles 16..31 alias slots 0..15
# shifted one frame.  Weight slot s holds k-tile PERM[s]: the first 16
# weight slots pair with x slots 0..7 (both plain and shifted), so the
# lo half of each GEMM group touches only lo-half tiles.
PERM = list(range(0, 8)) + list(range(16, 24)) + list(range(8, 16)) + list(range(24, 32))
_SLOT_OF = {t: s for s, t in enumerate(PERM)}
# matmul issue order: lo block (x slots 0..7), then hi block
KK_ORDER = [0, 1, 2, 3, 8, 9, 10, 11, 4, 5, 6, 7, 12, 13, 14, 15]

N_WARMUP = int(os.environ.get("KERNEL_N_WARMUP", "20"))


def _build_nc() -> bass.Bass:
    # Bacc (not bass.Bass): its compile() runs generate_event_semaphores(),
    # which splits multi-semaphore waits into event-sem chains — TRN2
    # instructions support at most one wait each.
    nc = bacc.Bacc("TRN2", target_bir_lowering=False, debug=False,
                   num_devices=N_CORES)
    dt = mybir.dt
    DR = mybir.MatmulPerfMode.DoubleRow

    xst_d = nc.dram_tensor("xst", [128, 16, 512], dt.float8e4,
                           kind="ExternalInput")
    xsp_d = nc.dram_tensor("xsp", [128, 16, 512], dt.float8e4,
                           kind="ExternalInput")
    wd = {}
    for trig in ("c", "s"):
        for ci, rows in enumerate(CHUNKS):
            nm = f"w{trig}{ci}"
            wd[nm] = nc.dram_tensor(nm, [128, 32, rows], dt.float8e4,
                                    kind="ExternalInput")
    out_d = nc.dram_tensor("out", [1, 1], dt.float32, kind="ExternalOutput")

    with ExitStack() as ctx:
        tc = ctx.enter_context(tile.TileContext(nc))
        xpool = ctx.enter_context(tc.tile_pool(name="x", bufs=1))
        wpool = ctx.enter_context(tc.tile_pool(name="w", bufs=1))
        pst_pool = ctx.enter_context(tc.tile_pool(name="pst", bufs=3, space="PSUM"))
        psp_pool = ctx.enter_context(tc.tile_pool(name="psp", bufs=2, space="PSUM"))
        ps1 = ctx.enter_context(tc.tile_pool(name="ps1", bufs=1, space="PSUM"))
        psb = ctx.enter_context(tc.tile_pool(name="psb", bufs=1, space="PSUM"))
        ptpool = ctx.enter_context(tc.tile_pool(name="pt", bufs=1))
        dpool = ctx.enter_context(tc.tile_pool(name="d", bufs=2))
        scpool = ctx.enter_context(tc.tile_pool(name="sc", bufs=3))
        stat = ctx.enter_context(tc.tile_pool(name="stat", bufs=1))

        # PE p-state warmup (see module docstring).  256-wide so the engine
        # time per matmul (213 ns at the mid p-state) exceeds the PE.SEQ
        # dispatch cost (~142 ns for ldweights+matmult) — narrower warmups
        # throttle on the sequencer and delay the first real GEMM dispatch.
        if N_WARMUP:
            wa = stat.tile([1, 256], dt.bfloat16)
            nc.vector.memset(wa[:, :], 1.0)
            wps = ps1.tile([128, 256], dt.float32)
            for _ in range(N_WARMUP):
                nc.tensor.matmul(wps[:, :], wa[:1, :128], wa[:1, :],
                                 start=True, stop=True)
            warm_junk = stat.tile([1, 1], dt.float32)
            nc.vector.tensor_copy(warm_junk[:1, :1], wps[0:1, 0:1])

        # lo/hi tiles: one DMA per tile
        xs = {}
        for nm, dram in (("t", xst_d), ("p", xsp_d)):
            xs[(nm, 0)] = xpool.tile([128, 8, 512], dt.float8e4,
                                     tag=f"xs{nm}lo", name=f"xs{nm}lo")
            xs[(nm, 1)] = xpool.tile([128, 8, 512], dt.float8e4,
                                     tag=f"xs{nm}hi", name=f"xs{nm}hi")
        wsb = {}
        for trig in ("c", "s"):
            for ci, rows in enumerate(CHUNKS):
                for half in (0, 1):
                    nm = f"w{trig}{ci}{'lo' if half == 0 else 'hi'}"
                    wsb[(trig, ci, half)] = wpool.tile(
                        [128, 16, rows], dt.float8e4, tag=nm, name=nm)

        def dma_w(trig, ci):
            rows = CHUNKS[ci]
            for half in (0, 1):
                nc.sync.dma_start(wsb[(trig, ci, half)][:, :, :],
                                  wd[f"w{trig}{ci}"][:, 16 * half:16 * half + 16, :])

        def dma_x(nm, dram):
            for half in (0, 1):
                nc.sync.dma_start(xs[(nm, half)][:, :, :],
                                  dram[:, 8 * half:8 * half + 8, :])

        # DMA order = PE need order (tgt groups c0..c3,s0..s3, then pred).
        dma_w("c", 0)
        dma_x("t", xst_d)
        dma_w("c", 1)
        dma_w("c", 2)
        dma_w("c", 3)
        dma_w("s", 0)
        dma_w("s", 1)
        dma_w("s", 2)
        dma_w("s", 3)
        dma_x("p", xsp_d)

        def gemm(ps_t, inp, trig, ci, f_lo=0, f_hi=NSEG, out0=0):
            """One PSD transform over frames [f_lo, f_hi) into psum columns
            [out0, out0 + f_hi - f_lo): 16 DoubleRow matmuls (32 k-tiles of
            128), lo-half tiles first."""
            rows = CHUNKS[ci]
            n = f_hi - f_lo
            for pos, kk in enumerate(KK_ORDER):
                t0 = 2 * kk
                s0 = _SLOT_OF[t0]
                xtile = xs[(inp, 0 if t0 % 16 < 8 else 1)]
                if t0 < 16:
                    rhs = xtile[:, (t0 % 8):(t0 % 8) + 2, f_lo:f_hi]
                else:
                    rhs = xtile[:, (t0 % 8):(t0 % 8) + 2, f_lo + 1:f_hi + 1]
                wtile = wsb[(trig, ci, 0 if s0 < 16 else 1)]
                nc.tensor.matmul(
                    ps_t[:rows, out0:out0 + n],
                    wtile[:, s0 % 16:s0 % 16 + 2, :rows],
                    rhs,
                    start=(pos == 0),
                    stop=(pos == 15),
                    perf_mode=DR,
                )

        GROUPS = [(t, c) for t in ("c", "s") for c in range(4)]

        # Phase 1: all 8 tgt transforms; each PSUM is copied to SBUF (pt_sb)
        # then squared+accumulated (Et) off the PE critical path.
        PT = {}
        E = {}
        for trig, ci in GROUPS:
            rows = CHUNKS[ci]
            ps_t = pst_pool.tile([128, NSEG], dt.float32, tag="ps_t")
            gemm(ps_t, "t", trig, ci)
            pt_sb = ptpool.tile([128, NSEG], dt.float32,
                                tag=f"pt_{trig}{ci}", name=f"pt_{trig}{ci}")
            nc.vector.tensor_copy(pt_sb[:rows, :], ps_t[:rows, :])
            PT[(trig, ci)] = pt_sb
            et = stat.tile([128, 1], dt.float32, tag=f"Et_{trig}{ci}",
                           name=f"Et_{trig}{ci}")
            tmp_t = scpool.tile([128, NSEG], dt.float32, tag="sq_t")
            nc.scalar.activation(
                out=tmp_t[:rows, :], in_=pt_sb[:rows, :],
                func=mybir.ActivationFunctionType.Square,
                accum_out=et[:rows, :])
            E[(1, trig, ci)] = et

        # Denominator sums + reciprocals (all off the critical path once
        # phase 1 is done).
        REC = {}
        for trig_unused in (None,):
            for ci, rows in enumerate(RCHUNKS):
                st_ = stat.tile([128, 1], dt.float32, tag=f"ST{ci}",
                                name=f"ST{ci}")
                rec = stat.tile([128, 1], dt.float32, tag=f"REC{ci}",
                                name=f"REC{ci}")
                nc.vector.tensor_add(st_[:rows, :], E[(1, "c", ci)][:rows, :],
                                     E[(1, "s", ci)][:rows, :])
                nc.vector.reciprocal(rec[:rows, :], st_[:rows, :])
                REC[ci] = rec

        # Phase 2: pred transforms; d = pt_sb - ps_p (one PSUM operand),
        # Er = sum_f d^2 via ACT Square+accum.  The very last group is
        # frame-split (448 + 63) into two PSUM groups on separate banks so
        # most of its sub runs while the PE finishes the 63-frame remainder
        # (shortens the tail-critical chain).
        FCUTS = [0, 384, 480, NSEG]
        for gi, (trig, ci) in enumerate(GROUPS):
            rows = CHUNKS[ci]
            last = (gi == len(GROUPS) - 1)
            ps_p = psp_pool.tile([128, NSEG], dt.float32, tag="ps_p")
            pt_sb = PT[(trig, ci)]
            d = dpool.tile([128, NSEG], dt.float32, tag="d")
            if last:
                # pieces land in separate PSUM banks (a matmul group start
                # zeroes its whole 2KB bank); the tgt pool is idle by now.
                ps_pb = psb.tile([128, FCUTS[2] - FCUTS[1]], dt.float32,
                                 tag="ps_pb", name="ps_pb")
                # same tag/shape as the (long-idle) tgt psum tiles so the
                # pst pool keeps its 1-bank buf size
                ps_pc = pst_pool.tile([128, NSEG], dt.float32,
                                      tag="ps_t", name="ps_pc")
                pieces = [ps_p, ps_pb, ps_pc]
                for (f0, f1), pp in zip(zip(FCUTS, FCUTS[1:]), pieces):
                    gemm(pp, "p", trig, ci, f0, f1, out0=0)
                for (f0, f1), pp in zip(zip(FCUTS, FCUTS[1:]), pieces):
                    nc.vector.tensor_sub(d[:rows, f0:f1],
                                         pt_sb[:rows, f0:f1],
                                         pp[:rows, :f1 - f0])
            else:
                gemm(ps_p, "p", trig, ci)
                nc.vector.tensor_sub(d[:rows, :], pt_sb[:rows, :],
                                     ps_p[:rows, :])
            er = stat.tile([128, 1], dt.float32, tag=f"Er_{trig}{ci}",
                           name=f"Er_{trig}{ci}")
            tmp_r = scpool.tile([128, NSEG], dt.float32, tag="sq_r")
            nc.scalar.activation(
                out=tmp_r[:rows, :], in_=d[:rows, :],
                func=mybir.ActivationFunctionType.Square,
                accum_out=er[:rows, :])
            E[(0, trig, ci)] = er

        # Finale: sum_bins (Er_c+Er_s) * rec as PE dot products accumulated
        # into one PSUM scalar (partition-dim reduce for free).
        tot = ps1.tile([1, 1], dt.float32)
        for ci, rows in enumerate(RCHUNKS):
            sr = stat.tile([128, 1], dt.float32, tag=f"SR{ci}", name=f"SR{ci}")
            nc.vector.tensor_add(sr[:rows, :], E[(0, "c", ci)][:rows, :],
                                 E[(0, "s", ci)][:rows, :])
            nc.tensor.matmul(tot[:1, :1], sr[:rows, :1], REC[ci][:rows, :1],
                             start=(ci == 0), stop=(ci == 3))
        red = stat.tile([1, 1], dt.float32)
        nc.vector.tensor_scalar_mul(red[:1, :1], tot[:1, :1], 2.0 / 480.0)
        nc.sync.dma_start(out_d[:, :], red[:1, :1])

    nc.compile()
    return nc


def _build_w():
    """fp8 DFT weights in the [part, slot, bin-chunk] layout:
      w{c,s}{ci}[p, s, n] = win[k] * {cos,sin}(2 pi k (21 + n0 + n) / 4096),
      k = 128 * PERM[s] + p.
    """
    k = np.arange(NPERSEG, dtype=np.float64)
    win = (0.5 - 0.5 * np.cos(2.0 * np.pi * k / NPERSEG)) * 2.0
    kb = np.arange(21, 21 + NBINS, dtype=np.float64)
    ang = 2.0 * np.pi * np.outer(k, kb) / NPERSEG
    nb_pad = sum(CHUNKS)
    C = np.zeros((NPERSEG, nb_pad), np.float32)
    S = np.zeros((NPERSEG, nb_pad), np.float32)
    C[:, :NBINS] = win[:, None] * np.cos(ang)
    S[:, :NBINS] = win[:, None] * np.sin(ang)
    out = {}
    for trig, M in (("c", C), ("s", S)):
        # [4096, 480] -> [p, j, n] -> permute j into slots
        M3 = M.reshape(32, 128, nb_pad).transpose(1, 0, 2)[:, PERM, :]
        col0 = 0
        for ci, rows in enumerate(CHUNKS):
            out[f"w{trig}{ci}"] = np.ascontiguousarray(
                M3[:, :, col0:col0 + rows]).astype(FP8)
            col0 += rows
    return out


_CACHE: dict = {}


def _get_prog():
    if "nc" not in _CACHE:
        _CACHE["nc"] = _build_nc()
    return _CACHE["nc"]


def _get_w():
    if "w" not in _CACHE:
        _CACHE["w"] = _build_w()
    return _CACHE["w"]


def _xs_layout(x: np.ndarray, core: int) -> np.ndarray:
    """[128 part, 16 slot, 512 grp] fp8 view of Welch row ROW0+core:
    xs[p, j, g] = R[2048 g + 128 j + p] where R is the row's 1048576
    samples (R[1024 b + m] = x[b, 1024 (ROW0+core) + m])."""
    c0 = (ROW0 + core) * 1024
    R = np.ascontiguousarray(x[:, c0:c0 + 1024]).reshape(-1).astype(FP8)
    return np.ascontiguousarray(R.reshape(512, 16, 128).transpose(2, 1, 0))


def kernel(pred: np.ndarray, target: np.ndarray, _trace: bool = False):
    nc = _get_prog()
    w = _get_w()
    pred = np.asarray(pred)
    target = np.asarray(target)
    in_maps = []
    for i in range(N_CORES):
        in_maps.append({
            "xst": _xs_layout(target, i),
            "xsp": _xs_layout(pred, i),
            **w,
        })
    res = run_bass_kernel_spmd(nc, in_maps, list(range(N_CORES)), trace=_trace)
    total = float(sum(float(res.results[i]["out"][0, 0])
                      for i in range(N_CORES)))
    out = np.array(total, dtype=np.float32)
    if _trace:
        return out, res
    return out
